# revision 20
# baseline (speedup 1.0000x reference)
"""Bass/Trainium2 kernel for nn_HALTON_33277406609678 (ragged_sequence).

Reference computation:
    feat[b] = max over compacted-valid positions p in [s_b, e_b] of
              (p-th valid token of enc[b] if p < num_valid_b else 0)
    out = relu(feat @ W1 + b1) @ W2 + b2

pos_span values live in [0, 40), so at most the first 48 (padded) valid
tokens of a row matter.  The host (cheap: indexing + dtype conversion
only) gathers those token rows per batch row into a dense fp16 tensor
laid out TRANSPOSED per D-chunk, so the device's span-max is a plain
strided reduce_max straight into the matmul's stationary layout -- no
indirect DMA, no PE transposes of gathered data.

Sharding: pure data parallel -- 8 batch rows per core, head weights
replicated (fp16).  b2 is added on the host (64x128 adds).

Slot semantics (host): slot j of row b holds compacted position q=s+j:
  real token       if q <= e and q <  nv
  zero row         if q <= e and q >= nv   (reference pools zeros there)
  dup of slot 0    if q >  e                (padding; never raises max)
If s >= nv the whole span is zero rows -> feat = 0 and the device MLP
yields relu(b1) @ W2 organically; no host patching needed.
"""

import numpy as np

B, L, D, H, K = 64, 512, 768, 768, 128
NCORES = 8
RPC = B // NCORES          # rows per core
CH = D // 128              # 128-wide chunks of D / H (= 6)

_CACHE = {}


def _build_nc(SLOTS):
    import concourse.bass as bass
    import concourse.bacc as bacc
    import concourse.mybir as mybir
    import concourse.tile as tile
    from concourse.masks import make_identity
    from contextlib import ExitStack

    f16 = mybir.dt.float16
    f32 = mybir.dt.float32

    nc = bacc.Bacc(
        "TRN2", target_bir_lowering=False, debug=False, num_devices=NCORES
    )
    GW = RPC * SLOTS  # gather cols per D-chunk
    CHH = CH // 2
    ga_d = nc.dram_tensor("ga", [128, CHH * GW], f16, kind="ExternalInput")
    gb_d = nc.dram_tensor("gb", [128, CHH * GW], f16, kind="ExternalInput")
    b1_d = nc.dram_tensor("b1", [128, CH], f32, kind="ExternalInput")
    w1_d = nc.dram_tensor("w1", [128, CH * H], f16, kind="ExternalInput")
    w2_d = nc.dram_tensor("w2", [128, CH * K], f16, kind="ExternalInput")
    out_d = nc.dram_tensor("out", [RPC, K], f16, kind="ExternalOutput")

    with tile.TileContext(nc) as tc, ExitStack() as ctx:
        cpool = ctx.enter_context(tc.tile_pool(name="const", bufs=1))
        spool = ctx.enter_context(tc.tile_pool(name="scratch", bufs=2))
        ppool_h = ctx.enter_context(tc.tile_pool(name="ph", bufs=1, space="PSUM"))
        ppool_t = ctx.enter_context(tc.tile_pool(name="pt", bufs=4, space="PSUM"))
        ppool_l = ctx.enter_context(tc.tile_pool(name="pl", bufs=1, space="PSUM"))

        # Gather split across BOTH HWDGE rings -- a single ring tops out well
        # under HBM rate, and the gather gates everything downstream.
        ga_sb = cpool.tile([128, CHH * GW], f16, tag="ga")
        nc.sync.dma_start(ga_sb[:], ga_d[:])
        gb_sb = cpool.tile([128, CHH * GW], f16, tag="gb")
        nc.scalar.dma_start(gb_sb[:], gb_d[:])

        # W1 as six single-chunk DMAs alternating rings in consumption order,
        # so chunk k lands ~in the order the matmuls consume it.  W2 last on
        # the scalar ring; tiny b1 rides the otherwise-idle SWDGE queue.
        w1_sb = cpool.tile([128, CH * H], f16, tag="w1")
        for kc in range(CH):
            eng = nc.sync if kc % 2 == 0 else nc.scalar
            eng.dma_start(w1_sb[:, kc * H:(kc + 1) * H], w1_d[:, kc * H:(kc + 1) * H])
        b1_sb = cpool.tile([128, CH], f32, tag="b1")
        nc.gpsimd.dma_start(b1_sb[:], b1_d[:])
        w2_sb = cpool.tile([128, CH * K], f16, tag="w2")
        nc.scalar.dma_start(w2_sb[:], w2_d[:])

        ident = cpool.tile([128, 128], f16, tag="ident")
        make_identity(nc, ident[:])

        # feat_c[d, r] = max over slots j of g[d, (c r j)]
        feat = []
        for c in range(CH):
            f = cpool.tile([128, RPC], f16, tag=f"feat{c}")
            half = ga_sb if c < CHH else gb_sb
            cc = c if c < CHH else c - CHH
            nc.vector.reduce_max(
                f[:],
                half[:, cc * GW:(cc + 1) * GW].rearrange("p (r j) -> p r j", j=SLOTS),
                axis=mybir.AxisListType.X,
            )
            feat.append(f)

        # h = feat @ W1 : [RPC, H] in two 384-wide PSUM halves; chunk-major
        # order so each W1 part unlocks its matmuls as it lands.
        NH = H // 2
        h_ps0 = ppool_h.tile([RPC, NH], f32, tag="h0")
        h_ps1 = ppool_h.tile([RPC, NH], f32, tag="h1")
        h_ps = [h_ps0, h_ps1]
        for kc in range(CH):
            for half in range(2):
                nc.tensor.matmul(
                    out=h_ps[half][:],
                    lhsT=feat[kc][:],
                    rhs=w1_sb[:, kc * H + half * NH: kc * H + (half + 1) * NH],
                    start=(kc == 0),
                    stop=(kc == CH - 1),
                )
        # PSUM -> fp16 SBUF per 128-chunk, then transpose -> relu(+b1) ->
        # logits matmul accumulate
        h_sb = spool.tile([RPC, H], f16, tag="hsb")
        for hc in range(CH):
            half, col = divmod(hc, CHH)
            nc.vector.tensor_copy(
                h_sb[:, hc * 128:(hc + 1) * 128],
                h_ps[half][:, col * 128:(col + 1) * 128])

        l_ps = ppool_l.tile([RPC, K], f32, tag="l")
        for hc in range(CH):
            ht_ps = ppool_t.tile([128, RPC], f16, tag="htp")
            nc.tensor.transpose(
                out=ht_ps[:], in_=h_sb[:, hc * 128:(hc + 1) * 128],
                identity=ident[:RPC, :RPC],
            )
            ht = spool.tile([128, RPC], f16, tag=f"ht{hc}")
            nc.vector.tensor_scalar(
                out=ht[:], in0=ht_ps[:], scalar1=b1_sb[:, hc:hc + 1], scalar2=0.0,
                op0=mybir.AluOpType.add, op1=mybir.AluOpType.max,
            )
            nc.tensor.matmul(
                out=l_ps[:],
                lhsT=ht[:],
                rhs=w2_sb[:, hc * K:(hc + 1) * K],
                start=(hc == 0),
                stop=(hc == CH - 1),
            )
        out_sb = spool.tile([RPC, K], f16, tag="out")
        nc.vector.tensor_copy(out_sb[:], l_ps[:])
        nc.sync.dma_start(out_d[:], out_sb[:])

    nc.compile()
    return nc


def _get_nc(slots):
    if slots not in _CACHE:
        _CACHE[slots] = _build_nc(slots)
    return _CACHE[slots]


def _pick_slots(valid_mask, pos_span):
    """Slot count for this input: longest span that can touch real/zero rows."""
    span = np.asarray(pos_span).astype(np.int64)
    ln = int((span[:, 1] - span[:, 0]).max()) + 1
    return max(8, (ln + 3) // 4 * 4)


def _host_gather(enc16, valid_mask, pos_span, SLOTS):
    """Dense [B, SLOTS] token values per the slot semantics above -> fp16."""
    v = np.asarray(valid_mask).astype(np.int64) == 1          # [B, L]
    span = np.asarray(pos_span).astype(np.int64)              # [B, 2]
    s, e = span[:, 0], span[:, 1]
    nv = v.sum(axis=1)                                        # num valid per row
    order = np.argsort(~v, axis=1, kind="stable")             # valid tokens first
    q = s[:, None] + np.arange(SLOTS)[None, :]                # rank per slot
    qc = np.where(q <= e[:, None], q, s[:, None])             # padding -> slot 0
    use_zero = qc >= nv[:, None]                              # [B, SLOTS]
    toks = np.take_along_axis(order, np.minimum(qc, L - 1), axis=1)
    vals = enc16[np.arange(B)[:, None], toks]                 # [B, SLOTS, D]
    vals[use_zero] = np.float16(0.0)
    return vals


def _make_in_maps(inputs):
    enc16 = np.asarray(inputs["encoder_layers"], dtype=np.float32).astype(np.float16)
    W1 = np.asarray(inputs["W1"], dtype=np.float32)
    b1 = np.asarray(inputs["b1"], dtype=np.float32)
    W2 = np.asarray(inputs["W2"], dtype=np.float32)

    SLOTS = _pick_slots(inputs["valid_mask"], inputs["pos_span"])
    vals = _host_gather(enc16, inputs["valid_mask"], inputs["pos_span"], SLOTS)

    # device layouts: partition = d % 128, free = (chunk, ...)
    w1_dev = np.ascontiguousarray(
        W1.astype(np.float16).reshape(CH, 128, H).transpose(1, 0, 2).reshape(128, CH * H))
    w2_dev = np.ascontiguousarray(
        W2.astype(np.float16).reshape(CH, 128, K).transpose(1, 0, 2).reshape(128, CH * K))
    b1_dev = np.ascontiguousarray(b1.reshape(CH, 128).T)      # [128, CH] f32

    in_maps = []
    half_cols = (CH // 2) * RPC * SLOTS
    for c in range(NCORES):
        rows = slice(c * RPC, (c + 1) * RPC)
        # g[d%128, (c r j)] = vals[r, j, d]
        g = (vals[rows]                                       # [RPC, SLOTS, D]
             .transpose(2, 0, 1)                              # [D, RPC, SLOTS]
             .reshape(CH, 128, RPC * SLOTS)
             .transpose(1, 0, 2)
             .reshape(128, CH * RPC * SLOTS))
        in_maps.append({
            "ga": np.ascontiguousarray(g[:, :half_cols]),
            "gb": np.ascontiguousarray(g[:, half_cols:]),
            "b1": b1_dev, "w1": w1_dev, "w2": w2_dev,
        })
    return in_maps, SLOTS


def _apply_compiler_flags():
    import os
    maxsem = os.environ.get("BASS_MAX_SEM_NUM")
    if not maxsem:
        return
    from concourse.compiler_utils import get_compiler_flags, set_compiler_flags
    flags = get_compiler_flags()
    if "--max-sem-num" not in flags:
        set_compiler_flags(flags + ["--max-sem-num", maxsem])


def kernel(**inputs):
    from concourse.bass_utils import run_bass_kernel_spmd

    _apply_compiler_flags()
    in_maps, slots = _make_in_maps(inputs)
    nc = _get_nc(slots)
    res = run_bass_kernel_spmd(nc, in_maps, list(range(NCORES)))
    out = np.concatenate(
        [res.results[c]["out"].astype(np.float32) for c in range(NCORES)], axis=0)

    b2 = np.asarray(inputs["b2"], dtype=np.float32)
    return (out + b2[None, :]).astype(np.float32)


# revision 22
# speedup vs baseline: 1.0804x; 1.0804x over previous
"""Bass/Trainium2 kernel for nn_HALTON_33277406609678 (ragged_sequence).

Reference computation:
    feat[b] = max over compacted-valid positions p in [s_b, e_b] of
              (p-th valid token of enc[b] if p < num_valid_b else 0)
    out = relu(feat @ W1 + b1) @ W2 + b2

pos_span values live in [0, 40), so at most the first 48 (padded) valid
tokens of a row matter.  The host (cheap: indexing + dtype conversion
only) gathers those token rows per batch row into a dense fp16 tensor
laid out TRANSPOSED per D-chunk, so the device's span-max is a plain
strided reduce_max straight into the matmul's stationary layout -- no
indirect DMA, no PE transposes of gathered data.

Sharding: pure data parallel -- 8 batch rows per core, head weights
replicated (fp16).  b2 is added on the host (64x128 adds).

Slot semantics (host): slot j of row b holds compacted position q=s+j:
  real token       if q <= e and q <  nv
  zero row         if q <= e and q >= nv   (reference pools zeros there)
  dup of slot 0    if q >  e                (padding; never raises max)
If s >= nv the whole span is zero rows -> feat = 0 and the device MLP
yields relu(b1) @ W2 organically; no host patching needed.
"""

import numpy as np

B, L, D, H, K = 64, 512, 768, 768, 128
NCORES = 8
RPC = B // NCORES          # rows per core
CH = D // 128              # 128-wide chunks of D / H (= 6)

_CACHE = {}


def _build_nc(SLOTS):
    import concourse.bass as bass
    import concourse.bacc as bacc
    import concourse.mybir as mybir
    import concourse.tile as tile
    from concourse.masks import make_identity
    from concourse.tile_rust import add_dep_helper
    from contextlib import ExitStack

    f16 = mybir.dt.float16
    f32 = mybir.dt.float32

    nc = bacc.Bacc(
        "TRN2", target_bir_lowering=False, debug=False, num_devices=NCORES
    )
    GW = RPC * SLOTS  # gather cols per D-chunk
    CHH = CH // 2
    ga_d = nc.dram_tensor("ga", [128, CHH * GW], f16, kind="ExternalInput")
    gb_d = nc.dram_tensor("gb", [128, CHH * GW], f16, kind="ExternalInput")
    b1_d = nc.dram_tensor("b1", [128, CH], f32, kind="ExternalInput")
    w1_d = nc.dram_tensor("w1", [128, CH * H], f16, kind="ExternalInput")
    w2_d = nc.dram_tensor("w2", [128, CH * K], f16, kind="ExternalInput")
    out_d = nc.dram_tensor("out", [RPC, K], f16, kind="ExternalOutput")

    with tile.TileContext(nc) as tc, ExitStack() as ctx:
        cpool = ctx.enter_context(tc.tile_pool(name="const", bufs=1))
        spool = ctx.enter_context(tc.tile_pool(name="scratch", bufs=2))
        ppool_h = ctx.enter_context(tc.tile_pool(name="ph", bufs=1, space="PSUM"))
        ppool_t = ctx.enter_context(tc.tile_pool(name="pt", bufs=4, space="PSUM"))
        ppool_l = ctx.enter_context(tc.tile_pool(name="pl", bufs=1, space="PSUM"))

        # Gather split across BOTH HWDGE rings -- a single ring tops out well
        # under HBM rate, and the gather gates everything downstream.
        ga_sb = cpool.tile([128, CHH * GW], f16, tag="ga")
        ga_i = nc.sync.dma_start(ga_sb[:], ga_d[:])
        gb_sb = cpool.tile([128, CHH * GW], f16, tag="gb")
        gb_i = nc.scalar.dma_start(gb_sb[:], gb_d[:])

        # W1 as six single-chunk DMAs alternating rings in consumption order,
        # so chunk k lands ~in the order the matmuls consume it.  W2 last on
        # the scalar ring; tiny b1 rides the otherwise-idle SWDGE queue.
        # Same-ring issue order is pinned with ordering deps: the ring drains
        # FIFO, so issue order IS bandwidth priority, and the scheduler would
        # otherwise shuffle it.
        w1_sb = cpool.tile([128, CH * H], f16, tag="w1")
        prev = {0: ga_i, 1: gb_i}
        for kc in range(CH):
            eng = nc.sync if kc % 2 == 0 else nc.scalar
            wi = eng.dma_start(w1_sb[:, kc * H:(kc + 1) * H],
                               w1_d[:, kc * H:(kc + 1) * H])
            add_dep_helper(wi.ins, prev[kc % 2].ins, sync=False,
                           reason="ring FIFO: consumption-order issue")
            prev[kc % 2] = wi
        b1_sb = cpool.tile([128, CH], f32, tag="b1")
        nc.gpsimd.dma_start(b1_sb[:], b1_d[:])
        w2_sb = cpool.tile([128, CH * K], f16, tag="w2")
        w2_i = nc.scalar.dma_start(w2_sb[:], w2_d[:])
        add_dep_helper(w2_i.ins, prev[1].ins, sync=False,
                       reason="ring FIFO: W2 after W1 chunks")

        ident = cpool.tile([128, 128], f16, tag="ident")
        make_identity(nc, ident[:])

        # feat_c[d, r] = max over slots j of g[d, (c r j)]
        feat = []
        for c in range(CH):
            f = cpool.tile([128, RPC], f16, tag=f"feat{c}")
            half = ga_sb if c < CHH else gb_sb
            cc = c if c < CHH else c - CHH
            nc.vector.reduce_max(
                f[:],
                half[:, cc * GW:(cc + 1) * GW].rearrange("p (r j) -> p r j", j=SLOTS),
                axis=mybir.AxisListType.X,
            )
            feat.append(f)

        # h = feat @ W1 : [RPC, H] in two 384-wide PSUM halves; chunk-major
        # order so each W1 part unlocks its matmuls as it lands.
        NH = H // 2
        h_ps0 = ppool_h.tile([RPC, NH], f32, tag="h0")
        h_ps1 = ppool_h.tile([RPC, NH], f32, tag="h1")
        h_ps = [h_ps0, h_ps1]
        for kc in range(CH):
            for half in range(2):
                nc.tensor.matmul(
                    out=h_ps[half][:],
                    lhsT=feat[kc][:],
                    rhs=w1_sb[:, kc * H + half * NH: kc * H + (half + 1) * NH],
                    start=(kc == 0),
                    stop=(kc == CH - 1),
                )
        # PSUM -> fp16 SBUF per 128-chunk, then transpose -> relu(+b1) ->
        # logits matmul accumulate
        h_sb = spool.tile([RPC, H], f16, tag="hsb")
        for hc in range(CH):
            half, col = divmod(hc, CHH)
            nc.vector.tensor_copy(
                h_sb[:, hc * 128:(hc + 1) * 128],
                h_ps[half][:, col * 128:(col + 1) * 128])

        l_ps = ppool_l.tile([RPC, K], f32, tag="l")
        for hc in range(CH):
            ht_ps = ppool_t.tile([128, RPC], f16, tag="htp")
            nc.tensor.transpose(
                out=ht_ps[:], in_=h_sb[:, hc * 128:(hc + 1) * 128],
                identity=ident[:RPC, :RPC],
            )
            ht = spool.tile([128, RPC], f16, tag=f"ht{hc}")
            nc.vector.tensor_scalar(
                out=ht[:], in0=ht_ps[:], scalar1=b1_sb[:, hc:hc + 1], scalar2=0.0,
                op0=mybir.AluOpType.add, op1=mybir.AluOpType.max,
            )
            nc.tensor.matmul(
                out=l_ps[:],
                lhsT=ht[:],
                rhs=w2_sb[:, hc * K:(hc + 1) * K],
                start=(hc == 0),
                stop=(hc == CH - 1),
            )
        out_sb = spool.tile([RPC, K], f16, tag="out")
        nc.vector.tensor_copy(out_sb[:], l_ps[:])
        nc.sync.dma_start(out_d[:], out_sb[:])

    nc.compile()
    return nc


def _get_nc(slots):
    if slots not in _CACHE:
        _CACHE[slots] = _build_nc(slots)
    return _CACHE[slots]


def _pick_slots(valid_mask, pos_span):
    """Slot count for this input: longest span that can touch real/zero rows."""
    span = np.asarray(pos_span).astype(np.int64)
    ln = int((span[:, 1] - span[:, 0]).max()) + 1
    return max(8, (ln + 3) // 4 * 4)


def _host_gather(enc16, valid_mask, pos_span, SLOTS):
    """Dense [B, SLOTS] token values per the slot semantics above -> fp16."""
    v = np.asarray(valid_mask).astype(np.int64) == 1          # [B, L]
    span = np.asarray(pos_span).astype(np.int64)              # [B, 2]
    s, e = span[:, 0], span[:, 1]
    nv = v.sum(axis=1)                                        # num valid per row
    order = np.argsort(~v, axis=1, kind="stable")             # valid tokens first
    q = s[:, None] + np.arange(SLOTS)[None, :]                # rank per slot
    qc = np.where(q <= e[:, None], q, s[:, None])             # padding -> slot 0
    use_zero = qc >= nv[:, None]                              # [B, SLOTS]
    toks = np.take_along_axis(order, np.minimum(qc, L - 1), axis=1)
    vals = enc16[np.arange(B)[:, None], toks]                 # [B, SLOTS, D]
    vals[use_zero] = np.float16(0.0)
    return vals


def _make_in_maps(inputs):
    enc16 = np.asarray(inputs["encoder_layers"], dtype=np.float32).astype(np.float16)
    W1 = np.asarray(inputs["W1"], dtype=np.float32)
    b1 = np.asarray(inputs["b1"], dtype=np.float32)
    W2 = np.asarray(inputs["W2"], dtype=np.float32)

    SLOTS = _pick_slots(inputs["valid_mask"], inputs["pos_span"])
    vals = _host_gather(enc16, inputs["valid_mask"], inputs["pos_span"], SLOTS)

    # device layouts: partition = d % 128, free = (chunk, ...)
    w1_dev = np.ascontiguousarray(
        W1.astype(np.float16).reshape(CH, 128, H).transpose(1, 0, 2).reshape(128, CH * H))
    w2_dev = np.ascontiguousarray(
        W2.astype(np.float16).reshape(CH, 128, K).transpose(1, 0, 2).reshape(128, CH * K))
    b1_dev = np.ascontiguousarray(b1.reshape(CH, 128).T)      # [128, CH] f32

    in_maps = []
    half_cols = (CH // 2) * RPC * SLOTS
    for c in range(NCORES):
        rows = slice(c * RPC, (c + 1) * RPC)
        # g[d%128, (c r j)] = vals[r, j, d]
        g = (vals[rows]                                       # [RPC, SLOTS, D]
             .transpose(2, 0, 1)                              # [D, RPC, SLOTS]
             .reshape(CH, 128, RPC * SLOTS)
             .transpose(1, 0, 2)
             .reshape(128, CH * RPC * SLOTS))
        in_maps.append({
            "ga": np.ascontiguousarray(g[:, :half_cols]),
            "gb": np.ascontiguousarray(g[:, half_cols:]),
            "b1": b1_dev, "w1": w1_dev, "w2": w2_dev,
        })
    return in_maps, SLOTS


def _apply_compiler_flags():
    import os
    maxsem = os.environ.get("BASS_MAX_SEM_NUM")
    if not maxsem:
        return
    from concourse.compiler_utils import get_compiler_flags, set_compiler_flags
    flags = get_compiler_flags()
    if "--max-sem-num" not in flags:
        set_compiler_flags(flags + ["--max-sem-num", maxsem])


def kernel(**inputs):
    from concourse.bass_utils import run_bass_kernel_spmd

    _apply_compiler_flags()
    in_maps, slots = _make_in_maps(inputs)
    nc = _get_nc(slots)
    res = run_bass_kernel_spmd(nc, in_maps, list(range(NCORES)))
    out = np.concatenate(
        [res.results[c]["out"].astype(np.float32) for c in range(NCORES)], axis=0)

    b2 = np.asarray(inputs["b2"], dtype=np.float32)
    return (out + b2[None, :]).astype(np.float32)
